# revision 58
# baseline (speedup 1.0000x reference)
"""Trainium2 Bass kernel for fused multi-head attention (16 heads, d=64,
b=2, n=2048, h=1024) across 8 NeuronCores.

Sharding: tensor-parallel over heads x data-parallel over batch.
Core c handles batch c//4 and heads [4*(c%4), 4*(c%4)+4). Each core
computes Q^T/K^T/V for its 4 heads over the full 2048-token sequence from
a replicated (per-batch) x. After attention, a small bf16 AllToAll per
512-row piece (Ulysses-style) swaps head-shards for row-shards, so each
core runs the output projection locally with the full 1024 contraction.

Differences vs the 328us baseline (now ~300us):
- K0/Q0 projections run with all 8 sc-accumulators live and the x-chunk
  (hk) loop outermost, so the PE consumes x as the three DMA queues
  deliver it; V projection and the pair-1 rotary are interleaved into
  the first attention pair instead of delaying attention start.
- Softmax exp is split per [128,1024] score tile: ScalarE computes
  cols 0:FE0 with the spline exp; VectorE computes cols FE0:1024 with a
  one-op Schraudolph fast-exp -- int16(EA*x+EB) written through a
  bitcast IS the bf16 exp approximation (bf16 = top 16 bits of f32).
  One softmax column = one (head, query), so numerator and denominator
  use the same approximation and the ~3% sawtooth error largely
  cancels; measured end-to-end error 1.1e-2 vs 7e-3 all-spline.
- Normalization: denominator rows and the AV accumulators are staged to
  SBUF right after each pair (freeing the PSUM banks), the PE
  broadcasts denominators, DVE computes reciprocals, and the final
  normalize-multiply runs as one partition-aligned op on the otherwise
  idle GpSimd engine.
- The last piece's AllToAll is split per head-pair: half 0 overlaps the
  tail of attention, halving the payload on the post-attention critical
  path. All mid-attention DMAs stay on the sync queue (DMAs issued from
  the scalar queue stall the exp stream behind them).
"""

import sys

if "/opt/trn_rl_repo" not in sys.path:
    sys.path.insert(0, "/opt/trn_rl_repo")

import numpy as np
import ml_dtypes

import concourse.bass as bass
import concourse.mybir as mybir
import concourse.tile as tile
from concourse import bacc
from concourse.bass import ts
from concourse.bass_utils import run_bass_kernel_spmd

BF16 = mybir.dt.bfloat16
F32 = mybir.dt.float32
I32 = mybir.dt.int32
I16 = mybir.dt.int16
ADD = mybir.AluOpType.add
MULT = mybir.AluOpType.mult
BYPASS = mybir.AluOpType.bypass
EXP = mybir.ActivationFunctionType.Exp

HEADS, D, H, N, B = 16, 64, 1024, 2048, 2
NC_ = 8
LH = 4            # local heads per core
LPAIRS = 2        # local head pairs
KC = 16           # k chunks of 128 over n=2048
QC = 4            # q chunks of 512 over n=2048 (= AllToAll pieces)
LVW = LH * 65     # 260: local v-aug width
LQK = LH * D      # 256 local q (or k) columns

# Schraudolph fast-exp in bf16 bit-space:
# exp(0.125*x) ~= bitcast_bf16(int16(EA*x + EB)),
# EA = 0.125 * 2^7/ln2, EB = 127*2^7 - C, C tuned for zero-mean ratio
# error (max ~3%).
EA = 0.125 * (2.0 ** 7) / float(np.log(2.0))
EB = float(127 * 2 ** 7 - 7.42)
FE0 = 704        # columns [FE0:1024) take the fast-exp path


def build_nc():
    nc = bacc.Bacc("TRN2", target_bir_lowering=False, debug=False, num_devices=NC_)

    xT = nc.declare_dram_parameter("xT", [H, N], BF16, isOutput=False)
    wqk = nc.declare_dram_parameter("wqk", [H, 2 * LQK], BF16, isOutput=False)
    wv = nc.declare_dram_parameter("wv", [H, LVW], BF16, isOutput=False)
    wout = nc.declare_dram_parameter("wout", [H, H], BF16, isOutput=False)
    cos2 = nc.declare_dram_parameter("cos2", [128, N], BF16, isOutput=False)
    # sswp[p] = sin value read at SOURCE partition p during the shuffle:
    # p%64 < 32 -> +sin[p%64+32], else -sin[p%64-32]
    sinm = nc.declare_dram_parameter("sinm", [128, N], BF16, isOutput=False)
    # msk[:,0]=1 iff this core's batch is 0; msk[:,1]=1 iff batch 1
    msk = nc.declare_dram_parameter("msk", [128, 2], F32, isOutput=False)
    out = nc.declare_dram_parameter("out", [QC, 128, H], F32, isOutput=True)

    with tile.TileContext(nc) as tc:
        with (
            tc.tile_pool(name="dram", bufs=1, space="DRAM") as dram,
            tc.tile_pool(name="sb", bufs=1) as sb,
            tc.tile_pool(name="sbw", bufs=1) as sbw,
            tc.tile_pool(name="psum", bufs=2, space="PSUM") as ps,
        ):
            a2a_in = [dram.tile([8, 2 * 128, 128], BF16, name=f"ain{i}")
                      for i in range(QC - 1)]
            a2a_out = [dram.tile([8, 2 * 128, 128], BF16, name=f"aout{i}")
                       for i in range(QC - 1)]
            # the last piece goes out as two half-piece AllToAlls (one per
            # head-pair): half 0 overlaps the tail of attention, half 1 is
            # all that remains on the critical path after the last norm
            a2a3_in = [dram.tile([8, 128, 128], BF16, name=f"a3in{p}")
                       for p in range(2)]
            a2a3_out = [dram.tile([8, 128, 128], BF16, name=f"a3out{p}")
                        for p in range(2)]

            warm_in = dram.tile([8, 128], BF16, name="warm_in")
            warm_out = dram.tile([8, 128], BF16, name="warm_out")
            warm_sb = sbw.tile([1, 128], BF16)
            nc.vector.memset(warm_sb[:, :], 0.0)
            nc.scalar.dma_start(warm_in[0:1, :], warm_sb[:, :])
            nc.gpsimd.dma_start(warm_in[1:2, :], warm_sb[:, :])
            nc.gpsimd.collective_compute(
                "AllToAll", BYPASS, replica_groups=[list(range(8))],
                ins=[warm_in.opt()], outs=[warm_out.opt()])
            # load the exp table during input staging, not at first score
            wexp = sbw.tile([1, 128], BF16)
            nc.scalar.activation(wexp[:, :], warm_sb[:, :], EXP)

            # ---- stage inputs; wqk first (needed by the hk-outer K proj),
            # x split across three DMA queues in hk order ----
            xt_sb = sbw.tile([128, 8 * N], BF16)
            wqk_sb = sbw.tile([128, 8 * 2 * LQK], BF16)
            wv_sb = sbw.tile([128, 8 * LVW], BF16)
            wout_sb = sbw.tile([128, 8 * H], BF16)
            cos2_sb = sbw.tile([128, N], BF16)
            sinm_sb = sbw.tile([128, N], BF16)
            ones_sb = sbw.tile([1, D], BF16)
            msk_sb = sbw.tile([128, 2], F32)
            nc.sync.dma_start(msk_sb[:, :], msk[:, :])
            # only sync/scalar/gpsimd can start DMAs, each ~90GB/s. Pack so
            # x chunks land in hk order ASAP (the hk-outer projection
            # consumes them), wqk leads on sync, sin/cos by ~20us (rotary),
            # wv by ~26us (V inside attention pair 0), wout much later.
            for hk in range(8):
                nc.sync.dma_start(wqk_sb[:, ts(hk, 2 * LQK)], wqk[ts(hk, 128), :])
            x_engs = [nc.scalar, nc.gpsimd, nc.scalar, nc.gpsimd,
                      nc.scalar, nc.sync, nc.gpsimd, nc.sync]
            for hk in range(8):
                x_engs[hk].dma_start(xt_sb[:, ts(hk, N)], xT[ts(hk, 128), :])
            nc.scalar.dma_start(sinm_sb[:, :], sinm[:, :])
            nc.scalar.dma_start(cos2_sb[:, :], cos2[:, :])
            for hk in range(8):
                nc.gpsimd.dma_start(wv_sb[:, ts(hk, LVW)], wv[ts(hk, 128), :])
            for hk in range(8):
                nc.scalar.dma_start(wout_sb[:, ts(hk, H)], wout[ts(hk, 128), :])
            nc.vector.memset(ones_sb[:, :], 1.0)

            kt_rot = sb.tile([128, 2 * N], BF16)   # [pair pr at pr*N][n]
            qt_rot = sb.tile([128, 2 * N], BF16)
            vt_all = sb.tile([128, KC * LVW], BF16)
            # attn^T laid out as [qc][row-block j][pair][row-in-block] so each
            # AllToAll shard (qc, j) is one contiguous 256-wide span
            attn_sb = sb.tile([128, 2 * N], BF16)
            attn4 = attn_sb.rearrange("p (q j r x) -> p q j r x", q=QC, j=4, r=2)

            def rotary_ops(stage, dst_ap):
                """Whole-pair rotary as a list of 6 deferred DVE ops (each
                ~1.2us), so callers can emit them where DVE has slack."""
                tmp = sb.tile([128, N], BF16, tag="rota", bufs=2, name="rota")
                tmp2 = sb.tile([128, N], BF16, tag="rotb", bufs=2, name="rotb")
                ops = []
                for hh in (0, 64):
                    ops.append(lambda hh=hh: nc.vector.tensor_tensor(
                        tmp[hh : hh + 32, :], stage[hh + 32 : hh + 64, :],
                        sinm_sb[hh + 32 : hh + 64, :], MULT))
                    ops.append(lambda hh=hh: nc.vector.tensor_tensor(
                        tmp[hh + 32 : hh + 64, :], stage[hh : hh + 32, :],
                        sinm_sb[hh : hh + 32, :], MULT))
                ops.append(lambda: nc.vector.tensor_tensor(
                    tmp2[:, :], stage[:, :], cos2_sb[:, :], MULT))
                ops.append(lambda: nc.vector.tensor_tensor(
                    dst_ap, tmp2[:, :], tmp[:, :], ADD))
                return ops

            def qk_pair_proj(pr):
                """K and Q projections for head-pair pr, all 8 sc-groups
                hk-outer (8 concurrent PSUM accumulators: K in the two
                [128,1024] tag-s slots, Q in tag-b + tag-av), so the PE
                consumes x chunks as the DMA delivers them."""
                pk = [ps.tile([128, 1024], F32, tag="s", name="pk")
                      for _ in range(2)]
                pq = [ps.tile([128, 512], F32, tag="b", name="pq")
                      for _ in range(2)]
                pq += [ps.tile([128, 512], F32, tag="av", name="pq2")
                       for _ in range(2)]
                for hk in range(8):
                    wK = wqk_sb[:, hk * 2 * LQK + LQK + pr * 128:][:, :128]
                    wQ = wqk_sb[:, hk * 2 * LQK + pr * 128:][:, :128]
                    for sc in range(4):
                        xr = xt_sb[:, hk * N + sc * 512:][:, :512]
                        nc.tensor.matmul(
                            pk[sc // 2][:, ts(sc % 2, 512)], lhsT=wK, rhs=xr,
                            start=(hk == 0), stop=(hk == 7))
                        nc.tensor.matmul(
                            pq[sc][:, :], lhsT=wQ, rhs=xr,
                            start=(hk == 0), stop=(hk == 7))
                stgK = sb.tile([128, N], BF16, tag="stg", bufs=2, name="stgK")
                stgQ = sb.tile([128, N], BF16, tag="stg", bufs=2, name="stgQ")
                for sc in range(4):
                    nc.scalar.copy(stgK[:, ts(sc, 512)],
                                   pk[sc // 2][:, ts(sc % 2, 512)])
                    nc.scalar.copy(stgQ[:, ts(sc, 512)], pq[sc][:, :])
                return (rotary_ops(stgK, kt_rot[:, pr * N:][:, :N])
                        + rotary_ops(stgQ, qt_rot[:, pr * N:][:, :N]))

            def v_group(g):
                """V projection for key-chunk g into vt_all[g] (8 matmuls +
                one staging copy); interleaved into the first attention
                pair's kc loop."""
                p = ps.tile([128, LVW], F32, tag="b", name="vp")
                for hk in range(8):
                    nc.tensor.matmul(
                        p[:, :],
                        lhsT=xt_sb[:, hk * N + g * 128:][:, :128],
                        rhs=wv_sb[:, ts(hk, LVW)],
                        start=(hk == 0),
                        stop=(hk == 7),
                    )
                nc.scalar.copy(
                    vt_all[:, ts(g, LVW)].rearrange(
                        "p (h e) -> p h e", e=65)[:, :, 0:64],
                    p.rearrange("p (h e) -> p h e", e=65)[:, :, 0:64])

            nc.vector.memset(
                vt_all.rearrange("p (g e) -> p g e", e=65)[:, :, 64:65], 1.0)
            rot0 = qk_pair_proj(0)
            # pair-0 rotary runs on DVE while the PE does pair-1 projections
            for op in rot0:
                op()
            rot1 = qk_pair_proj(1)  # drained inside attention pair (0,0)

            # ---- attention; per-piece AllToAll + local output projection ----
            def emit_rd(st):
                """Right after the pair's last AV matmul: copy out the two
                denominator rows, and stage the AV accumulators to SBUF
                (head b to partitions 64:128) so the PSUM banks free early
                and the norm multiply is one partition-aligned GpSimd op."""
                av0, av1, qc, pr = st
                dn = sb.tile([1, 1024], BF16, tag="dn", bufs=2, name="dn")
                nc.vector.tensor_copy(dn[:, 0:512], av0[64:65, :])
                nc.vector.tensor_copy(dn[:, 512:1024], av1[64:65, :])
                stg = sb.tile([128, 512], F32, tag="avst", bufs=2, name="avst")
                nc.vector.tensor_copy(stg[0:64, :], av0[0:64, :])
                nc.vector.tensor_copy(stg[64:128, :], av1[0:64, :])
                return (dn, stg)

            def emit_bcast(dn):
                b_ps = ps.tile([128, 512], F32, tag="b", name="b_ps")
                nc.tensor.matmul(b_ps[0:64, :], lhsT=ones_sb[:, :],
                                 rhs=dn[:, 0:512], start=True, stop=True,
                                 tile_position=(0, 0))
                nc.tensor.matmul(b_ps[64:128, :], lhsT=ones_sb[:, :],
                                 rhs=dn[:, 512:1024], start=True, stop=True,
                                 tile_position=(0, 64))
                return b_ps

            def emit_bd(b_ps):
                bd_sb = sb.tile([128, 512], F32, tag="bsd", bufs=2,
                                name="bd_sb")
                nc.scalar.copy(bd_sb[:, :], b_ps[:, :])
                return bd_sb

            def emit_recip(bd_sb):
                b_sb = sb.tile([128, 512], F32, tag="bsb", bufs=2, name="b_sb")
                nc.vector.reciprocal_approx_fast(out=b_sb[:, :],
                                                 in_=bd_sb[:, :])
                return b_sb

            def emit_normmul(st, stg, b_sb):
                _, _, qc, pr = st
                dst = attn4[:, qc, :, pr, :]  # [128, 4, 128]
                b3 = b_sb.rearrange("p (j x) -> p j x", x=128)
                nc.gpsimd.tensor_tensor(
                    dst, stg.rearrange("p (j x) -> p j x", x=128), b3, MULT)

            def emit_a2a(qc):
                # shard j of a2a_in = my 2 head-pair chunks for row block
                # j%4, duplicated to both batch groups (receiver masks off
                # the cross-batch half)
                for j in range(8):
                    nc.sync.dma_start(
                        a2a_in[qc][j].rearrange("(r p) x -> p r x", p=128),
                        attn4[:, qc, j % 4, :, :])
                nc.gpsimd.collective_compute(
                    "AllToAll", BYPASS, replica_groups=[list(range(8))],
                    ins=[a2a_in[qc].opt()], outs=[a2a_out[qc].opt()])

            def emit_a2a_half(prh):
                for j in range(8):
                    nc.sync.dma_start(a2a3_in[prh][j],
                                      attn4[:, 3, j % 4, prh, :])
                nc.gpsimd.collective_compute(
                    "AllToAll", BYPASS, replica_groups=[list(range(8))],
                    ins=[a2a3_in[prh].opt()], outs=[a2a3_out[prh].opt()])

            def emit_outproj(qc, tail=False):
                # raw slots from all 8 ranks, then mask-combine batch halves;
                # combine(hc) then its two matmuls so the PE pipelines behind
                # the DVE instead of waiting for all 16 combine ops
                att_r = sb.tile([128, 16 * 128], BF16, tag="attr", bufs=2,
                                name="att_r")
                r3 = att_r.rearrange("p (c x) -> p c x", x=128)
                for i in range(8):
                    if tail == 2:
                        eng = nc.scalar
                    else:
                        eng = nc.scalar if (tail and i % 2) else nc.sync
                    if qc == 3:
                        eng.dma_start(r3[:, 2 * i, :], a2a3_out[0][i])
                        eng.dma_start(r3[:, 2 * i + 1, :], a2a3_out[1][i])
                    else:
                        eng.dma_start(
                            r3[:, 2 * i : 2 * i + 2, :],
                            a2a_out[qc][i].rearrange("(c p) x -> p c x",
                                                     p=128))
                att_g = sb.tile([128, 8 * 128], BF16, tag="attg", bufs=2,
                                name="att_g")
                g3 = att_g.rearrange("p (c x) -> p c x", x=128)
                tmpm = sb.tile([128, 2 * 128], BF16, tag="tmpm", bufs=2,
                               name="tmpm")
                o_ps = [ps.tile([128, 512], F32, tag="b", name="o_ps")
                        for _ in range(2)]
                for hc in range(8):
                    lo = r3[:, (hc // 2) * 2 + (hc % 2), :]
                    hi = r3[:, 8 + (hc // 2) * 2 + (hc % 2), :]
                    tm = tmpm[:, ts(hc % 2, 128)]
                    nc.vector.tensor_scalar_mul(tm, hi, msk_sb[:, 1:2])
                    nc.vector.scalar_tensor_tensor(
                        g3[:, hc, :], lo, msk_sb[:, 0:1], tm, MULT, ADD)
                    for nh in range(2):
                        nc.tensor.matmul(
                            o_ps[nh][:, :],
                            lhsT=g3[:, hc, :],
                            rhs=wout_sb[:, hc * H + nh * 512:][:, :512],
                            start=(hc == 0),
                            stop=(hc == 7),
                        )
                for nh in range(2):
                    ob = sb.tile([128, 512], F32, tag="ob", bufs=3, name="ob")
                    nc.vector.tensor_copy(ob[:, :], o_ps[nh][:, :])
                    if tail:
                        nc.sync.dma_start(out[qc, 0:64, ts(nh, 512)],
                                          ob[0:64, :])
                        nc.scalar.dma_start(out[qc, 64:128, ts(nh, 512)],
                                            ob[64:128, :])
                    else:
                        nc.sync.dma_start(out[qc, :, ts(nh, 512)], ob[:, :])

            norm_pending = None   # (av0, av1, qc, pr)
            norm_dn = None
            norm_bsb = None
            a2a_ready = []        # pieces normalized, awaiting A2A emission
            for qc in range(QC):
                for pr in range(LPAIRS):
                    qt_p = qt_rot[:, pr * N + qc * 512:][:, :512]
                    av0 = ps.tile([65, 512], F32, tag="av", name="av0")
                    av1 = ps.tile([65, 512], F32, tag="av", name="av1")
                    exps = []
                    for kc in range(KC):
                        s_ps = ps.tile([128, 1024], F32, tag="s", name="s_ps")
                        nc.tensor.matmul(
                            s_ps[:, 0:512],
                            lhsT=kt_rot[0:64, pr * N + kc * 128:][:, :128],
                            rhs=qt_p[0:64, :], start=True, stop=True,
                            tile_position=(0, 0))
                        nc.tensor.matmul(
                            s_ps[:, 512:1024],
                            lhsT=kt_rot[64:128, pr * N + kc * 128:][:, :128],
                            rhs=qt_p[64:128, :], start=True, stop=True,
                            tile_position=(64, 0))
                        e = sb.tile([128, 1024], BF16, tag="exp", bufs=4,
                                    name="e")
                        nc.scalar.activation(e[:, 0:FE0], s_ps[:, 0:FE0], EXP,
                                             scale=0.125)
                        if FE0 < 1024:
                            # bf16 = top 16 bits of f32, so int16(EA*x+EB)
                            # written into the e-tile IS the bf16 fast-exp
                            nc.vector.tensor_scalar(
                                e[:, FE0:1024].bitcast(I16), s_ps[:, FE0:1024],
                                EA, EB, MULT, ADD)
                        exps.append(e)
                        if qc == 0 and pr == 0:
                            v_group(kc)
                            if kc < 12:
                                rot1[kc]()
                        if kc == 1 and norm_pending is not None:
                            norm_bsb = emit_recip(emit_bd(emit_bcast(
                                norm_dn[0])))
                        if kc == 3 and norm_pending is not None:
                            emit_normmul(norm_pending, norm_dn[1], norm_bsb)
                            if norm_pending[3] == 1:  # piece complete
                                a2a_ready.append(norm_pending[2])
                            norm_pending = None
                        if kc == 10 and pr == 0 and a2a_ready:
                            emit_a2a(a2a_ready.pop(0))
                        if kc == 10 and pr == 1 and qc == 3:
                            emit_a2a_half(0)
                        if kc == 8 and qc == 3:
                            emit_outproj(pr)  # pieces 0 and 1
                        if kc > 0:
                            _av_mm(nc, vt_all, exps[kc - 1], av0, av1,
                                   kc - 1, pr)
                    _av_mm(nc, vt_all, exps[KC - 1], av0, av1, KC - 1, pr)
                    norm_pending = (av0, av1, qc, pr)
                    norm_dn = emit_rd(norm_pending)
            norm_bsb = emit_recip(emit_bd(emit_bcast(norm_dn[0])))
            emit_normmul(norm_pending, norm_dn[1], norm_bsb)
            emit_a2a_half(1)
            emit_outproj(QC - 2, tail=True)
            emit_outproj(QC - 1, tail=True)

    nc.finalize()
    return nc


def _av_mm(nc, vt_all, e, av0, av1, kc, pr):
    nc.tensor.matmul(
        av0[:, :], lhsT=vt_all[:, kc * LVW + 65 * (2 * pr):][:, :65],
        rhs=e[:, 0:512], start=(kc == 0), stop=(kc == KC - 1))
    nc.tensor.matmul(
        av1[:, :], lhsT=vt_all[:, kc * LVW + 65 * (2 * pr + 1):][:, :65],
        rhs=e[:, 512:1024], start=(kc == 0), stop=(kc == KC - 1))


_NC = None


def _get_nc():
    global _NC
    if _NC is None:
        _NC = build_nc()
    return _NC


def _bf16(a):
    return np.ascontiguousarray(a.astype(ml_dtypes.bfloat16))


def make_in_maps(x, rotary_emb, w_qkv, w_out):
    x = np.asarray(x, np.float32)
    rotary_emb = np.asarray(rotary_emb, np.float32)
    w_qkv = np.asarray(w_qkv, np.float32)
    w_out = np.asarray(w_out, np.float32)
    cosT = np.cos(rotary_emb).T.astype(np.float32)  # [64, N]
    sinT = np.sin(rotary_emb).T.astype(np.float32)
    cos2_a = _bf16(np.concatenate([cosT, cosT], axis=0))
    sswp = np.concatenate([sinT[32:], -sinT[:32]], axis=0)
    sinm_a = _bf16(np.concatenate([sswp, sswp], axis=0))
    wout_bf = _bf16(w_out)
    in_maps = []
    for c in range(NC_):
        b, hb = c // 4, c % 4
        h0 = LH * hb
        wq_loc = w_qkv[:, 64 * h0 : 64 * h0 + LQK]
        wk_loc = w_qkv[:, H + 64 * h0 : H + 64 * h0 + LQK]
        wv_loc = w_qkv[:, 2 * H + 64 * h0 : 2 * H + 64 * h0 + LQK]
        wv_aug = np.zeros((H, LVW), np.float32)
        for j in range(LH):
            wv_aug[:, 65 * j : 65 * j + 64] = wv_loc[:, 64 * j : 64 * j + 64]
        msk_a = np.zeros((128, 2), np.float32)
        msk_a[:, b] = 1.0
        in_maps.append({
            "xT": _bf16(x[b].T),
            "msk": msk_a,
            "wqk": _bf16(np.concatenate([wq_loc, wk_loc], axis=1)),
            "wv": _bf16(wv_aug),
            "wout": wout_bf,
            "cos2": cos2_a,
            "sinm": sinm_a,
        })
    return in_maps


def run(x, rotary_emb, w_qkv, w_out, trace=False, tmpdir=None):
    nc = _get_nc()
    in_maps = make_in_maps(x, rotary_emb, w_qkv, w_out)
    res = run_bass_kernel_spmd(nc, in_maps, list(range(NC_)), trace=trace,
                               tmpdir=tmpdir)
    full = np.empty((B, N, H), np.float32)
    for c in range(NC_):
        b, r = c // 4, c % 4
        piece = np.asarray(res.results[c]["out"], np.float32)  # [QC, 128, H]
        for qc in range(QC):
            full[b, 512 * qc + 128 * r : 512 * qc + 128 * r + 128] = piece[qc]
    return full, res


def kernel(x, rotary_emb, w_qkv, w_out):
    full, _ = run(x, rotary_emb, w_qkv, w_out)
    return full
